# revision 1
# baseline (speedup 1.0000x reference)
"""MoE layer (B=8192, D=2048, H=2048, E=8, top-2) on 8 TRN2 NeuronCores.

Strategy: expert-parallel with host-side routing (the "all-to-all tokens by
routed expert" sharding). kernel() receives the FULL inputs on host, so the
dispatch/combine all-to-all is simply the sharding step:

  1. Gating (0.2% of FLOPs) on host with jax-CPU, bit-matching the
     reference's `x @ gate_W.T + gate_b` -> top_k -> softmax.
  2. For each expert e, gather its routed tokens (~B*K/E = 2048 of them),
     pad to a common capacity C, and hand core e the pair
     (xT_e [D, C], wT_e = expert_W[e].T [D, H]) in bf16.
  3. Each core computes Y_e = X_e @ W_e.T (fp32 accumulate) -- this is
     99.2% of the model FLOPs and 4x less work than the dense einsum.
  4. Host combine: out[b] = sum_k w_k[b] * (Y_{e_k(b)}[col(b)] + b_{e_k(b)}).
"""

import math

import numpy as np

B, D, H, E, TOPK = 8192, 2048, 2048, 8, 2
NCORES = 8

# test.py flips TRACE to profile HW exec time; grading leaves it False.
TRACE = False
# Hand-rolled segment matmul (_build_bass_custom) measured 261-281us vs
# 257-263us for matmul_tile_kernel + warm-up; kept for reference, disabled.
CUSTOM = False
last_exec_time_ns = None
last_trace_path = None


def _routing(x, gate_W, gate_b):
    """Reference-exact gating on jax-CPU: logits -> top_k -> softmax."""
    import jax
    import jax.numpy as jnp

    with jax.default_device(jax.devices("cpu")[0]):
        logits = jnp.asarray(x) @ jnp.asarray(gate_W).T + jnp.asarray(gate_b)
        topk_vals, topk_idx = jax.lax.top_k(logits, TOPK)
        topk_w = jax.nn.softmax(topk_vals, axis=1)
    return np.asarray(topk_idx), np.asarray(topk_w, dtype=np.float32)


def _build_bass(seg_rows):
    """One Bass program, SPMD across cores. For segment sizes [s_0..s_{P-1}]
    (summing to C), computes y[off_j:off_j+s_j] = xT[:, off_j:...].T @ w_j
    with a per-core weight tensor per segment."""
    import concourse.bacc as bacc
    import concourse.mybir as mybir
    import concourse.tile as tile
    from concourse.kernels.tile_matmul import matmul_tile_kernel

    C = sum(seg_rows)
    nc = bacc.Bacc("TRN2", target_bir_lowering=False)
    xT = nc.dram_tensor("xT", [D, C], mybir.dt.bfloat16, kind="ExternalInput")
    ws = [
        nc.dram_tensor(f"w{j}", [D, H], mybir.dt.bfloat16, kind="ExternalInput")
        for j in range(len(seg_rows))
    ]
    y = nc.dram_tensor("y", [C, H], mybir.dt.float32, kind="ExternalOutput")
    with tile.TileContext(nc) as tc:
        # PE warm-up: ~100 tiny matmuls with no DMA deps run during the
        # initial tile-fill window, tripping the HAM activity monitor
        # (4096-cycle window) so the real matmuls start at 2.4 GHz instead
        # of the cold 1.2 GHz, and bridging the idle gap so it can't
        # re-throttle before the first real matmul (~13us in).
        with (
            tc.tile_pool(name="warm", bufs=1) as warm,
            tc.tile_pool(name="warmp", bufs=1, space="PSUM") as warmp,
        ):
            # DVE memset (nc.any routes memset to gpsimd, whose dispatch
            # delayed the first dummy ~2us; Tile requires the tile be
            # written before the matmuls read it).
            wa = warm.tile([128, 128], mybir.dt.bfloat16)
            nc.vector.memset(wa[:], 0.0)
            # Rotate psum banks so back-to-back dummies pipeline at issue
            # rate instead of serializing on one bank's drain.
            pts = [
                warmp.tile([128, 128], mybir.dt.float32, name=f"wp{i}", tag=f"wp{i}")
                for i in range(4)
            ]
            # 48 dummies from ~5.5us end ~10.5us: HAM trips ~2us earlier
            # than the memset version (margin against its +-3.4us phase
            # jitter) and the idle gap to the first real matmul (~12.7us)
            # stays under the 3.4us re-throttle window.
            for i in range(48):
                nc.tensor.matmul(pts[i % 4][:], wa[:], wa[:], start=True, stop=True)
        off = 0
        for j, s in enumerate(seg_rows):
            matmul_tile_kernel(
                tc,
                xT[:, off : off + s],
                ws[j][:],
                y[off : off + s, :],
                # DVE psum evictions (267ns) instead of the default ACT
                # ACTIVATE (687ns): mid-stream they're hidden either way,
                # but the last few serialize after the final matmul.
                psum_evict_fn=lambda nc, psum, sbuf: nc.vector.tensor_copy(
                    out=sbuf, in_=psum
                ),
            )
            off += s
    nc.compile()
    return nc


def _build_bass_custom(seg_rows):
    """Hand-rolled replacement for matmul_tile_kernel: same math, but DMA
    sizes/order tuned so the first matmul starts ~8us in (512KB first chunks),
    weights stay SBUF-resident per segment (double-buffered across segments),
    and PSUM evictions go through the vector engine."""
    import concourse.bacc as bacc
    import concourse.mybir as mybir
    import concourse.tile as tile

    P, NB = 128, H // 512          # 128 partitions, 4 n-chunks of 512
    KO = D // P                    # 16 contraction sub-tiles
    KG = 4                         # ko-group size per weight DMA (512KB chunks)
    NG = KO // KG
    bf16, f32 = mybir.dt.bfloat16, mybir.dt.float32

    C = sum(seg_rows)
    nc = bacc.Bacc("TRN2", target_bir_lowering=False)
    xT = nc.dram_tensor("xT", [D, C], bf16, kind="ExternalInput")
    ws = [
        nc.dram_tensor(f"w{j}", [D, H], bf16, kind="ExternalInput")
        for j in range(len(seg_rows))
    ]
    y = nc.dram_tensor("y", [C, H], f32, kind="ExternalOutput")

    with tile.TileContext(nc) as tc:
        with (
            tc.tile_pool(name="warm", bufs=1) as warm,
            tc.tile_pool(name="warmp", bufs=1, space="PSUM") as warmp,
            tc.tile_pool(name="wpool", bufs=2) as wpool,
            tc.tile_pool(name="xpool", bufs=2) as xpool,
            tc.tile_pool(name="ypool", bufs=3) as ypool,
            tc.tile_pool(name="psum", bufs=2, space="PSUM") as psum,
        ):
            # PE warm-up across the DMA-fill window (see _build_bass).
            wa = warm.tile([P, P], bf16)
            nc.any.memset(wa[:], 0.0)
            pts = [
                warmp.tile([P, P], f32, name=f"wp{i}", tag=f"wp{i}") for i in range(4)
            ]
            for i in range(40):
                nc.tensor.matmul(pts[i % 4][:], wa[:], wa[:], start=True, stop=True)

            xr = xT.rearrange("(ko p) s -> p ko s", p=P)
            off = 0
            for j, S in enumerate(seg_rows):
                wr = ws[j].rearrange("(ko p) h -> p ko h", p=P)
                M = S // P
                xt = [None] * M  # x m-chunks, loaded in the first n-sweep, then resident
                # n-outer: only x[m0] + w[n0,g0] (1MB) gate the first matmul;
                # other weight chunks stream behind 58us-per-n of compute.
                # x loads issue on sync, weight prefetches on gpsimd, so the
                # two don't serialize on one issue queue.
                for n in range(NB):
                    if n == 0:
                        xt[0] = xpool.tile([P, KO, P], bf16, name=f"x{j}_0", tag="x_0")
                        nc.sync.dma_start(xt[0][:], xr[:, :, off : off + P])
                    wt = []
                    for g in range(NG):
                        t = wpool.tile(
                            [P, KG, 512], bf16, name=f"w{j}_{n}_{g}", tag=f"w_{g}"
                        )
                        nc.sync.dma_start(
                            t[:],
                            wr[:, g * KG : (g + 1) * KG, n * 512 : (n + 1) * 512],
                        )
                        wt.append(t)
                    for m in range(M):
                        if n == 0 and m > 0:
                            xt[m] = xpool.tile(
                                [P, KO, P], bf16, name=f"x{j}_{m}", tag=f"x_{m}"
                            )
                            nc.sync.dma_start(
                                xt[m][:], xr[:, :, off + m * P : off + (m + 1) * P]
                            )
                        ps = psum.tile([P, 512], f32, tag="ps")
                        for ko in range(KO):
                            nc.tensor.matmul(
                                ps[:],
                                xt[m][:, ko, :],
                                wt[ko // KG][:, ko % KG, :],
                                start=(ko == 0),
                                stop=(ko == KO - 1),
                            )
                        yt = ypool.tile([P, 512], f32, tag="y")
                        nc.vector.tensor_copy(yt[:], ps[:])
                        nc.sync.dma_start(
                            y[
                                off + m * P : off + (m + 1) * P,
                                n * 512 : (n + 1) * 512,
                            ],
                            yt[:],
                        )
                off += S
    nc.compile()
    return nc


def _plan_segments(counts):
    """Choose per-core segment row-sizes (same across cores) and assign every
    expert's token blocks to (core, segment) pieces.

    Returns (seg_rows, pieces) where pieces[e] = ordered [(core, seg, rows)]
    covering counts[e] rows, and no (core, seg) holds more than one expert.
    Falls back to one max-capacity segment per core when the balanced packing
    doesn't fit.
    """
    blocks = [-(-int(n) // 128) for n in counts]
    total = sum(blocks)
    T = -(-total // 8)

    # Candidate per-core block splits: every segment's row count must keep a
    # large M_TILE (divisible by 384 or 512 -> block counts div by 3 or 4).
    def ok(b):
        return b > 0 and (b % 3 == 0 or b % 4 == 0)

    schemes = []
    if ok(T):
        schemes.append([T])
    schemes += [[b1, T - b1] for b1 in range(T - 1, 0, -1) if ok(b1) and ok(T - b1)]

    for seg_blocks in schemes:
        pool = []  # (blocks_capacity, core, seg)
        for c in range(8):
            for j, b in enumerate(seg_blocks):
                pool.append([b, c, j])
        pieces = [[] for _ in range(E)]
        feasible = True
        # Largest experts first; take largest segments first.
        for e in sorted(range(E), key=lambda e: -blocks[e]):
            need = blocks[e]
            while need > 0:
                pool.sort(key=lambda s: -s[0])
                if not pool or pool[0][0] == 0:
                    feasible = False
                    break
                # Prefer an exact fit, else the largest.
                pick = next((s for s in pool if s[0] == need), pool[0])
                take = min(pick[0], need)
                pieces[e].append((pick[1], pick[2], take * 128))
                need -= take
                pool.remove(pick)
            if not feasible:
                break
        if feasible:
            seg_rows = [b * 128 for b in seg_blocks]
            # Trim the last piece of each expert to its true row count.
            for e in range(E):
                used = sum(p[2] for p in pieces[e])
                over = used - int(counts[e])
                if over > 0:
                    c, j, r = pieces[e][-1]
                    pieces[e][-1] = (c, j, r - over)
            return seg_rows, pieces

    # Fallback: single segment of max capacity (always feasible).
    mx = max(512, int(counts.max()))
    C = min(math.ceil(mx / 384) * 384, math.ceil(mx / 512) * 512)
    return [C], [[(e, 0, int(counts[e]))] for e in range(E)]


def _install_profshim():
    """Register the NTFF profile hook trn_boot couldn't (image's antenv lacks
    axon_hooks) and stub the S3 artifact upload. Only needed when TRACE."""
    import sys
    import types

    import antenv

    if "antenv.axon_hooks" not in sys.modules:
        mod = types.ModuleType("antenv.axon_hooks")
        _hook = [None]
        mod.set_axon_ntff_profile_hook = lambda h: _hook.__setitem__(0, h)
        mod.get_axon_ntff_profile_hook = lambda: _hook[0]
        sys.modules["antenv.axon_hooks"] = mod
        antenv.axon_hooks = mod
        from trn_agent_boot.trn_boot import _ntff_profile_via_ctypes

        mod.set_axon_ntff_profile_hook(
            _ntff_profile_via_ctypes("/opt/axon/libaxon_pjrt.so")
        )
    import concourse.bass_utils as _bu

    _bu.upload_artifacts = lambda tmpdir: f"local:{tmpdir}"


def kernel(x, expert_W, expert_b, gate_W, gate_b):
    global last_exec_time_ns, last_trace_path
    import ml_dtypes

    from concourse.bass_utils import run_bass_kernel_spmd

    x = np.asarray(x, dtype=np.float32)
    expert_W = np.asarray(expert_W, dtype=np.float32)
    expert_b = np.asarray(expert_b, dtype=np.float32)
    gate_W = np.asarray(gate_W, dtype=np.float32)
    gate_b = np.asarray(gate_b, dtype=np.float32)

    topk_idx, topk_w = _routing(x, gate_W, gate_b)

    # Dispatch: token lists per expert (each token appears in exactly TOPK lists).
    tok = [np.nonzero((topk_idx == e).any(axis=1))[0] for e in range(E)]
    counts = np.array([len(t) for t in tok])
    seg_rows, pieces = _plan_segments(counts)
    seg_off = np.concatenate([[0], np.cumsum(seg_rows)])
    C = int(seg_off[-1])

    bf16 = ml_dtypes.bfloat16
    xb = x.astype(bf16)  # one RTN cast, reused for all gathers
    wb = [np.ascontiguousarray(expert_W[e].T.astype(bf16)) for e in range(E)]

    # Dispatch per the plan: fill each core's xT columns and pick per-segment
    # weights; record each token's (core, row) for the combine.
    xTs = [np.zeros((D, C), dtype=bf16) for _ in range(NCORES)]
    seg_w = [[0] * len(seg_rows) for _ in range(NCORES)]  # expert id per slot
    core_of = np.zeros((E, B), dtype=np.int64)
    pos_of = np.zeros((E, B), dtype=np.int64)
    for e in range(E):
        cum = 0
        for c, j, rows in pieces[e]:
            t = tok[e][cum : cum + rows]
            lo = int(seg_off[j])
            xTs[c][:, lo : lo + len(t)] = xb[t].T
            seg_w[c][j] = e
            core_of[e, t] = c
            pos_of[e, t] = lo + np.arange(len(t))
            cum += rows

    in_maps = []
    for c in range(NCORES):
        m = {"xT": xTs[c]}
        for j in range(len(seg_rows)):
            m[f"w{j}"] = wb[seg_w[c][j]]
        in_maps.append(m)

    if TRACE:
        _install_profshim()
    nc = _build_bass_custom(seg_rows) if CUSTOM else _build_bass(seg_rows)
    res = run_bass_kernel_spmd(nc, in_maps, list(range(NCORES)), trace=TRACE)
    last_exec_time_ns = res.exec_time_ns
    if res.instructions_and_trace:
        last_trace_path = res.instructions_and_trace[1]

    Ys = np.stack([res.results[c]["y"] for c in range(NCORES)])  # [8, C, H]

    # Combine: out[b] = sum_k w_k * (Y at (core,row of (e_k, b)) + b_{e_k})
    barange = np.arange(B)
    out = np.zeros((B, H), dtype=np.float32)
    for k in range(TOPK):
        ek = topk_idx[:, k]
        out += topk_w[:, k, None] * (
            Ys[core_of[ek, barange], pos_of[ek, barange], :] + expert_b[ek]
        )
    return out



# revision 3
# speedup vs baseline: 1.0384x; 1.0384x over previous
"""MoE layer (B=8192, D=2048, H=2048, E=8, top-2) on 8 TRN2 NeuronCores.

Strategy: expert-parallel with host-side routing + PER-PAIR MIXED PRECISION.
kernel() receives the FULL inputs on host, so the dispatch/combine all-to-all
is simply the sharding step:

  1. Gating (0.2% of FLOPs) on host with jax-CPU, bit-matching the
     reference's `x @ gate_W.T + gate_b` -> top_k -> softmax.
  2. Each (token, expert) pair is assigned a precision by its gate weight:
     pairs with w > t (~the top-1 pairs) run in bf16, pairs with w <= t run
     in fp8 e4m3 with MatmulPerfMode.DoubleRow (2x PE throughput, 157 TF/s).
     Host-simulated rel-err of this split is ~1.7e-2 vs the 2e-2 gate
     (plain-fp8 everything would be 3.3e-2 — fails).
  3. Per (expert, precision) group, gather routed tokens, pad to 128-row
     blocks (bf16 groups are instead PROMOTED: filled with the highest-w fp8
     pairs, which improves error at equal cost), balance weighted cost
     (bf16 row = 2 units, fp8 row = 1 unit) across the 8 cores.
  4. Each core computes Y_seg = X_seg @ W_seg.T (fp32 PSUM accumulate,
     bf16 output) for its segments: fp8 segments first (halves the DMA
     bytes gating the first matmul), then bf16.
  5. Host combine: out[b] = sum_k w_k[b] * (Y_{pair}[row(pair)]/scale(pair)
     + b_{e_k(b)}), where scale folds the fp8 quantization scaling 2^-15.
"""

import math

import numpy as np

B, D, H, E, TOPK = 8192, 2048, 2048, 8, 2
NCORES = 8

# Fraction of (token, expert) pairs quantized to fp8, by gate-weight quantile.
# Q=0.5 <=> threshold 0.5 <=> top-2 pairs fp8 (host-sim rel err 1.71e-2).
Q_FP8 = 0.50
SX = 2.0**4   # x fp8 scale: randn * 16 -> well inside e4m3 normal range
SW = 2.0**11  # W fp8 scale: U(+-0.0221) * 2048 -> +-45, top e4m3 octaves
WARMUP_MM = 48

# test.py flips TRACE to profile HW exec time; grading leaves it False.
TRACE = False
last_exec_time_ns = None
last_trace_path = None


def _routing(x, gate_W, gate_b):
    """Reference-exact gating on jax-CPU: logits -> top_k -> softmax."""
    import jax
    import jax.numpy as jnp

    with jax.default_device(jax.devices("cpu")[0]):
        logits = jnp.asarray(x) @ jnp.asarray(gate_W).T + jnp.asarray(gate_b)
        topk_vals, topk_idx = jax.lax.top_k(logits, TOPK)
        topk_w = jax.nn.softmax(topk_vals, axis=1)
    return np.asarray(topk_idx), np.asarray(topk_w, dtype=np.float32)


def _plan_precision(topk_idx, topk_w):
    """Split each expert's pairs into a bf16 group (high gate weight) and an
    fp8 group (low), with bf16 group sizes exact multiples of 128 via
    promotion, and the global bf16 block count a multiple of 8.

    Returns per-expert (tokens_bf, ks_bf, tokens_f8, ks_f8) row lists, each
    bf list exactly m_e*128 long.
    """
    t = np.quantile(topk_w.flatten(), Q_FP8)
    pair_lists = []
    m_blocks = np.zeros(E, np.int64)
    for e in range(E):
        bb, kk = np.nonzero(topk_idx == e)
        ww = topk_w[bb, kk]
        o = np.argsort(-ww, kind="stable")
        bb, kk, ww = bb[o], kk[o], ww[o]
        pair_lists.append((bb, kk))
        nb = int((ww > t).sum())
        m_blocks[e] = min(-(-nb // 128), len(ww) // 128)
    # promote whole blocks (expert with most fp8 rows left) until the bf16
    # block total is a multiple of 8 -> per-core bf16 rows identical with no
    # bf16 padding at all.
    while m_blocks.sum() % 8:
        cand = max(range(E), key=lambda e: len(pair_lists[e][0]) - m_blocks[e] * 128)
        m_blocks[cand] += 1
    groups = []
    for e in range(E):
        bb, kk = pair_lists[e]
        cut = int(m_blocks[e]) * 128
        groups.append((bb[:cut], kk[:cut], bb[cut:], kk[cut:]))
    return groups


def _plan_segments(counts, nseg_cap=2):
    """Choose per-core segment row-sizes (same across cores) and assign every
    group's token blocks to (core, segment) pieces.

    Returns (seg_rows, pieces) where pieces[e] = ordered [(core, seg, rows)]
    covering counts[e] rows, and no (core, seg) holds more than one group.
    """
    blocks = [-(-int(n) // 128) for n in counts]
    total = sum(blocks)
    if total == 0:
        return [], [[] for _ in counts]
    T = -(-total // 8)

    # Candidate per-core block splits: every segment's row count must keep a
    # large M_TILE (divisible by 384 or 512 -> block counts div by 3 or 4).
    def ok(b):
        return b > 0 and (b % 3 == 0 or b % 4 == 0)

    schemes = []
    if ok(T):
        schemes.append([T])
    schemes += [[b1, T - b1] for b1 in range(T - 1, 0, -1) if ok(b1) and ok(T - b1)]

    for seg_blocks in schemes:
        pool = []  # (blocks_capacity, core, seg)
        for c in range(8):
            for j, b in enumerate(seg_blocks):
                pool.append([b, c, j])
        pieces = [[] for _ in counts]
        feasible = True
        # Largest groups first; take largest segments first.
        for e in sorted(range(len(counts)), key=lambda e: -blocks[e]):
            need = blocks[e]
            while need > 0:
                pool.sort(key=lambda s: -s[0])
                if not pool or pool[0][0] == 0:
                    feasible = False
                    break
                # Prefer an exact fit, else the largest.
                pick = next((s for s in pool if s[0] == need), pool[0])
                take = min(pick[0], need)
                pieces[e].append((pick[1], pick[2], take * 128))
                need -= take
                pool.remove(pick)
            if not feasible:
                break
        if feasible:
            seg_rows = [b * 128 for b in seg_blocks]
            # Trim the last piece of each group to its true row count.
            for e in range(len(counts)):
                used = sum(p[2] for p in pieces[e])
                over = used - int(counts[e])
                if over > 0:
                    c, j, r = pieces[e][-1]
                    pieces[e][-1] = (c, j, r - over)
            return seg_rows, pieces

    # Fallback: single segment of max capacity (always feasible).
    mx = max(512, int(max(counts)))
    C = min(math.ceil(mx / 384) * 384, math.ceil(mx / 512) * 512)
    return [C], [[(e, 0, int(counts[e]))] for e in range(len(counts))]


def _build_bass(seg8_rows, segb_rows):
    """One Bass program, SPMD across cores. fp8 (DoubleRow) segments first,
    then bf16 segments. y rows: [fp8 rows..., bf16 rows...], bf16 dtype."""
    import concourse.bacc as bacc
    import concourse.mybir as mybir
    import concourse.tile as tile
    from concourse.kernels.tile_matmul import matmul_tile_kernel

    C8, Cb = sum(seg8_rows), sum(segb_rows)
    nc = bacc.Bacc("TRN2", target_bir_lowering=False)
    xT8 = (
        nc.dram_tensor("xT8", [D, C8], mybir.dt.float8e4, kind="ExternalInput")
        if C8
        else None
    )
    xTb = (
        nc.dram_tensor("xTb", [D, Cb], mybir.dt.bfloat16, kind="ExternalInput")
        if Cb
        else None
    )
    w8s = [
        nc.dram_tensor(f"w8_{j}", [D, H], mybir.dt.float8e4, kind="ExternalInput")
        for j in range(len(seg8_rows))
    ]
    wbs = [
        nc.dram_tensor(f"wb_{j}", [D, H], mybir.dt.bfloat16, kind="ExternalInput")
        for j in range(len(segb_rows))
    ]
    y = nc.dram_tensor("y", [C8 + Cb, H], mybir.dt.bfloat16, kind="ExternalOutput")
    with tile.TileContext(nc) as tc:
        # PE warm-up: tiny matmuls with no DMA deps run during the initial
        # tile-fill window, tripping the HAM activity monitor (4096-cycle
        # window) so the real matmuls start at 2.4 GHz instead of the cold
        # 1.2 GHz, and bridging the idle gap so it can't re-throttle before
        # the first real matmul.
        with (
            tc.tile_pool(name="warm", bufs=1) as warm,
            tc.tile_pool(name="warmp", bufs=1, space="PSUM") as warmp,
        ):
            wa = warm.tile([128, 128], mybir.dt.bfloat16)
            nc.vector.memset(wa[:], 0.0)
            # Rotate psum banks so back-to-back dummies pipeline at issue
            # rate instead of serializing on one bank's drain.
            pts = [
                warmp.tile([128, 128], mybir.dt.float32, name=f"wp{i}", tag=f"wp{i}")
                for i in range(4)
            ]
            for i in range(WARMUP_MM):
                nc.tensor.matmul(pts[i % 4][:], wa[:], wa[:], start=True, stop=True)

        evict = lambda nc, psum, sbuf: nc.vector.tensor_copy(out=sbuf, in_=psum)
        off = 0
        for j, s in enumerate(seg8_rows):
            matmul_tile_kernel(
                tc,
                xT8[:, off : off + s],
                w8s[j][:],
                y[off : off + s, :],
                psum_evict_fn=evict,
            )
            off += s
        for j, s in enumerate(segb_rows):
            matmul_tile_kernel(
                tc,
                xTb[:, off - C8 : off - C8 + s],
                wbs[j][:],
                y[off : off + s, :],
                psum_evict_fn=evict,
            )
            off += s
    nc.compile()
    return nc


def _install_profshim():
    """Register the NTFF profile hook trn_boot couldn't (image's antenv lacks
    axon_hooks) and stub the S3 artifact upload. Only needed when TRACE."""
    import sys
    import types

    import antenv

    if "antenv.axon_hooks" not in sys.modules:
        mod = types.ModuleType("antenv.axon_hooks")
        _hook = [None]
        mod.set_axon_ntff_profile_hook = lambda h: _hook.__setitem__(0, h)
        mod.get_axon_ntff_profile_hook = lambda: _hook[0]
        sys.modules["antenv.axon_hooks"] = mod
        antenv.axon_hooks = mod
        from trn_agent_boot.trn_boot import _ntff_profile_via_ctypes

        mod.set_axon_ntff_profile_hook(
            _ntff_profile_via_ctypes("/opt/axon/libaxon_pjrt.so")
        )
    import concourse.bass_utils as _bu

    _bu.upload_artifacts = lambda tmpdir: f"local:{tmpdir}"


def kernel(x, expert_W, expert_b, gate_W, gate_b):
    global last_exec_time_ns, last_trace_path
    import ml_dtypes

    from concourse.bass_utils import run_bass_kernel_spmd

    x = np.asarray(x, dtype=np.float32)
    expert_W = np.asarray(expert_W, dtype=np.float32)
    expert_b = np.asarray(expert_b, dtype=np.float32)
    gate_W = np.asarray(gate_W, dtype=np.float32)
    gate_b = np.asarray(gate_b, dtype=np.float32)

    topk_idx, topk_w = _routing(x, gate_W, gate_b)
    groups = _plan_precision(topk_idx, topk_w)

    counts_b = np.array([len(g[0]) for g in groups])
    counts_8 = np.array([len(g[2]) for g in groups])
    seg8_rows, pieces8 = _plan_segments(counts_8)
    segb_rows, piecesb = _plan_segments(counts_b)
    seg8_off = np.concatenate([[0], np.cumsum(seg8_rows)]).astype(int)
    segb_off = np.concatenate([[0], np.cumsum(segb_rows)]).astype(int)
    C8, Cb = int(seg8_off[-1]), int(segb_off[-1])

    bf16 = ml_dtypes.bfloat16
    f8 = ml_dtypes.float8_e4m3
    xb = x.astype(bf16)  # one RTN cast, reused for all bf16 gathers
    x8 = (x * np.float32(SX)).astype(f8)
    wb = [np.ascontiguousarray(expert_W[e].T.astype(bf16)) for e in range(E)]
    w8 = [
        np.ascontiguousarray((expert_W[e].T * np.float32(SW)).astype(f8))
        for e in range(E)
    ]

    # Dispatch per the plans: fill each core's xT8/xTb columns and pick
    # per-segment weights; record each pair's (core, y-row) for the combine.
    xT8s = [np.zeros((D, C8), dtype=f8) for _ in range(NCORES)]
    xTbs = [np.zeros((D, Cb), dtype=bf16) for _ in range(NCORES)]
    seg8_w = [[0] * len(seg8_rows) for _ in range(NCORES)]
    segb_w = [[0] * len(segb_rows) for _ in range(NCORES)]
    core_of = np.zeros((E, B), dtype=np.int64)
    pos_of = np.zeros((E, B), dtype=np.int64)
    is8_of = np.zeros((E, B), dtype=bool)
    for e in range(E):
        bb_b, _kb, bb_8, _k8 = groups[e]
        cum = 0
        for c, j, rows in piecesb[e]:
            tkn = bb_b[cum : cum + rows]
            lo = int(segb_off[j])
            xTbs[c][:, lo : lo + len(tkn)] = xb[tkn].T
            segb_w[c][j] = e
            core_of[e, tkn] = c
            pos_of[e, tkn] = C8 + lo + np.arange(len(tkn))
            cum += rows
        cum = 0
        for c, j, rows in pieces8[e]:
            tkn = bb_8[cum : cum + rows]
            lo = int(seg8_off[j])
            xT8s[c][:, lo : lo + len(tkn)] = x8[tkn].T
            seg8_w[c][j] = e
            core_of[e, tkn] = c
            pos_of[e, tkn] = lo + np.arange(len(tkn))
            is8_of[e, tkn] = True
            cum += rows

    in_maps = []
    for c in range(NCORES):
        m = {}
        if C8:
            m["xT8"] = xT8s[c]
        if Cb:
            m["xTb"] = xTbs[c]
        for j in range(len(seg8_rows)):
            m[f"w8_{j}"] = w8[seg8_w[c][j]]
        for j in range(len(segb_rows)):
            m[f"wb_{j}"] = wb[segb_w[c][j]]
        in_maps.append(m)

    if TRACE:
        _install_profshim()
    nc = _build_bass(seg8_rows, segb_rows)
    res = run_bass_kernel_spmd(nc, in_maps, list(range(NCORES)), trace=TRACE)
    last_exec_time_ns = res.exec_time_ns
    if res.instructions_and_trace:
        last_trace_path = res.instructions_and_trace[1]

    Ys = np.stack([res.results[c]["y"] for c in range(NCORES)]).astype(np.float32)

    # Combine: out[b] = sum_k w_k * (Y at (core,row of (e_k, b))/scale + b_e)
    barange = np.arange(B)
    descale = np.float32(1.0 / (SX * SW))
    out = np.zeros((B, H), dtype=np.float32)
    for k in range(TOPK):
        ek = topk_idx[:, k]
        wk = topk_w[:, k]
        yv = Ys[core_of[ek, barange], pos_of[ek, barange], :]
        sc = np.where(is8_of[ek, barange], wk * descale, wk).astype(np.float32)
        out += sc[:, None] * yv + wk[:, None] * expert_b[ek]
    return out


# revision 4
# speedup vs baseline: 1.2245x; 1.1792x over previous
"""MoE layer (B=8192, D=2048, H=2048, E=8, top-2) on 8 TRN2 NeuronCores.

Strategy: expert-parallel with host-side routing + PER-PAIR MIXED PRECISION.
kernel() receives the FULL inputs on host, so the dispatch/combine all-to-all
is simply the sharding step:

  1. Gating (0.2% of FLOPs) on host with jax-CPU, bit-matching the
     reference's `x @ gate_W.T + gate_b` -> top_k -> softmax.
  2. Per expert, its T_BF*128 highest-gate-weight (token, expert) pairs run
     in bf16; the rest run in fp8 e4m3 with MatmulPerfMode.DoubleRow (2x PE
     throughput). Host-sim rel-err of this split is 1.77e-2 vs the 2e-2
     gate (plain-fp8 everything would be 3.3e-2 — fails; all-bf16 is the
     roofline-2x-slower baseline). fp8 scales are chosen to align the
     uniform W distribution with the e4m3 grid (~15% less quant error than
     naive power-of-2 scaling); the descale folds into the host combine.
  3. bf16 side: core e computes expert e's T_BF*128 pairs — zero padding.
     fp8 side: per-expert remainders are split into 128-row-block pieces
     and best-fit packed into identical per-core segments (SPMD: all cores
     share one program, so segment shapes must match across cores).
  4. Each core runs one matmul_tile_kernel call per segment (fp8 segments
     first: their first tiles are half the bytes, so the PE starts ~earlier;
     fp32 PSUM accumulate, bf16 output evicted via the vector engine).
  5. Host combine: out[b] = sum_k w_k[b] * (Y_pair[row(pair)]/scale(pair)
     + b_{e_k(b)}).
"""

import numpy as np

B, D, H, E, TOPK = 8192, 2048, 2048, 8, 2
NCORES = 8

T_BF = 8          # bf16 blocks (x128 rows) per expert = per core
SX = 24.0         # x fp8 scale (randn -> +-131, e4m3 normal range)
SW = 2976.0       # W fp8 scale (U(+-0.0221) -> +-65.8, grid-aligned)
WARMUP_MM = 48

# test.py flips TRACE to profile HW exec time; grading leaves it False.
TRACE = False
last_exec_time_ns = None
last_trace_path = None


def _routing(x, gate_W, gate_b):
    """Reference-exact gating on jax-CPU: logits -> top_k -> softmax."""
    import jax
    import jax.numpy as jnp

    with jax.default_device(jax.devices("cpu")[0]):
        logits = jnp.asarray(x) @ jnp.asarray(gate_W).T + jnp.asarray(gate_b)
        topk_vals, topk_idx = jax.lax.top_k(logits, TOPK)
        topk_w = jax.nn.softmax(topk_vals, axis=1)
    return np.asarray(topk_idx), np.asarray(topk_w, dtype=np.float32)


def _split_precision(topk_idx, topk_w):
    """Per expert: the T_BF*128 highest-w pairs -> bf16, rest -> fp8.
    Returns per-expert (tokens_bf, tokens_f8) sorted by w desc."""
    groups = []
    for e in range(E):
        bb, kk = np.nonzero(topk_idx == e)
        ww = topk_w[bb, kk]
        o = np.argsort(-ww, kind="stable")
        bb = bb[o]
        cut = T_BF * 128
        groups.append((bb[:cut], bb[cut:]))
        assert len(bb) >= cut, f"expert {e} has only {len(bb)} pairs"
    return groups


def _ok(b):
    # Segment block counts divisible by 3 or 4 keep matmul_tile_kernel's
    # M_TILE at 384/512 (vs 128), avoiding extra W re-streaming DMA.
    return b > 0 and (b % 3 == 0 or b % 4 == 0)


def _pack_fp8(counts):
    """Pack per-expert fp8 row counts into identical per-core segments.

    Returns (seg_rows, pieces): seg_rows is the per-core segment row sizes
    (same on every core); pieces[e] = [(core, seg, rows)] covering counts[e],
    each (core, seg) slot holding at most one expert.
    """
    blocks = [-(-int(n) // 128) for n in counts]
    total = sum(blocks)
    if total == 0:
        return [], [[] for _ in counts]

    t0 = -(-total // 8)
    for T8 in range(t0, t0 + 5):
        schemes = []
        if _ok(T8):
            schemes.append([T8])
        schemes += [
            [a, T8 - a] for a in range(T8 - 1, T8 // 2 - 1, -1) if _ok(a) and _ok(T8 - a)
        ]
        for seg_blocks in schemes:
            bins = []  # [capacity_blocks, core, seg]
            for c in range(8):
                for j, bcap in enumerate(seg_blocks):
                    bins.append([bcap, c, j])
            pieces = [[] for _ in counts]
            feasible = True
            for e in sorted(range(len(counts)), key=lambda e: -blocks[e]):
                rem = blocks[e]
                while rem > 0 and bins:
                    bins.sort(key=lambda s: -s[0])
                    if rem >= bins[0][0]:
                        pick = bins[0]  # fill the largest bin completely
                        take = pick[0]
                    else:
                        # best fit: smallest bin that holds the remainder
                        pick = min(
                            (s for s in bins if s[0] >= rem), key=lambda s: s[0]
                        )
                        take = rem
                    pieces[e].append((pick[1], pick[2], take * 128))
                    rem -= take
                    bins.remove(pick)
                if rem > 0:
                    feasible = False
                    break
            if feasible:
                seg_rows = [b * 128 for b in seg_blocks]
                # Trim each expert's last piece to its true row count.
                for e in range(len(counts)):
                    used = sum(p[2] for p in pieces[e])
                    over = used - int(counts[e])
                    if over > 0:
                        c, j, r = pieces[e][-1]
                        pieces[e][-1] = (c, j, r - over)
                return seg_rows, pieces
    raise RuntimeError("fp8 packing failed")


def _build_bass(seg8_rows, nbf_rows):
    """One Bass program, SPMD across cores. fp8 (DoubleRow) segments first,
    then the single bf16 segment. y rows: [fp8..., bf16...], bf16 dtype."""
    import concourse.bacc as bacc
    import concourse.mybir as mybir
    import concourse.tile as tile
    from concourse.kernels.tile_matmul import matmul_tile_kernel

    C8 = sum(seg8_rows)
    C = C8 + nbf_rows
    nc = bacc.Bacc("TRN2", target_bir_lowering=False)
    xT8 = nc.dram_tensor("xT8", [D, C8], mybir.dt.float8e4, kind="ExternalInput")
    xTb = nc.dram_tensor("xTb", [D, nbf_rows], mybir.dt.bfloat16, kind="ExternalInput")
    w8s = [
        nc.dram_tensor(f"w8_{j}", [D, H], mybir.dt.float8e4, kind="ExternalInput")
        for j in range(len(seg8_rows))
    ]
    wb = nc.dram_tensor("wb", [D, H], mybir.dt.bfloat16, kind="ExternalInput")
    y = nc.dram_tensor("y", [C, H], mybir.dt.bfloat16, kind="ExternalOutput")
    with tile.TileContext(nc) as tc:
        # PE warm-up: tiny matmuls with no DMA deps run during the initial
        # tile-fill window, tripping the HAM activity monitor (4096-cycle
        # window) so the real matmuls start at 2.4 GHz instead of the cold
        # 1.2 GHz, and bridging the idle gap so it can't re-throttle before
        # the first real matmul.
        with (
            tc.tile_pool(name="warm", bufs=1) as warm,
            tc.tile_pool(name="warmp", bufs=1, space="PSUM") as warmp,
        ):
            wa = warm.tile([128, 128], mybir.dt.bfloat16)
            nc.vector.memset(wa[:], 0.0)
            pts = [
                warmp.tile([128, 128], mybir.dt.float32, name=f"wp{i}", tag=f"wp{i}")
                for i in range(4)
            ]
            for i in range(WARMUP_MM):
                nc.tensor.matmul(pts[i % 4][:], wa[:], wa[:], start=True, stop=True)

        evict = lambda nc, psum, sbuf: nc.vector.tensor_copy(out=sbuf, in_=psum)
        off = 0
        for j, s in enumerate(seg8_rows):
            matmul_tile_kernel(
                tc,
                xT8[:, off : off + s],
                w8s[j][:],
                y[off : off + s, :],
                psum_evict_fn=evict,
            )
            off += s
        matmul_tile_kernel(
            tc,
            xTb[:, :],
            wb[:],
            y[C8:, :],
            psum_evict_fn=evict,
        )
    nc.compile()
    return nc


def _install_profshim():
    """Register the NTFF profile hook trn_boot couldn't (image's antenv lacks
    axon_hooks) and stub the S3 artifact upload. Only needed when TRACE."""
    import sys
    import types

    import antenv

    if "antenv.axon_hooks" not in sys.modules:
        mod = types.ModuleType("antenv.axon_hooks")
        _hook = [None]
        mod.set_axon_ntff_profile_hook = lambda h: _hook.__setitem__(0, h)
        mod.get_axon_ntff_profile_hook = lambda: _hook[0]
        sys.modules["antenv.axon_hooks"] = mod
        antenv.axon_hooks = mod
        from trn_agent_boot.trn_boot import _ntff_profile_via_ctypes

        mod.set_axon_ntff_profile_hook(
            _ntff_profile_via_ctypes("/opt/axon/libaxon_pjrt.so")
        )
    import concourse.bass_utils as _bu

    _bu.upload_artifacts = lambda tmpdir: f"local:{tmpdir}"


def kernel(x, expert_W, expert_b, gate_W, gate_b):
    global last_exec_time_ns, last_trace_path
    import ml_dtypes

    from concourse.bass_utils import run_bass_kernel_spmd

    x = np.asarray(x, dtype=np.float32)
    expert_W = np.asarray(expert_W, dtype=np.float32)
    expert_b = np.asarray(expert_b, dtype=np.float32)
    gate_W = np.asarray(gate_W, dtype=np.float32)
    gate_b = np.asarray(gate_b, dtype=np.float32)

    topk_idx, topk_w = _routing(x, gate_W, gate_b)
    groups = _split_precision(topk_idx, topk_w)

    counts_8 = np.array([len(g[1]) for g in groups])
    seg8_rows, pieces8 = _pack_fp8(counts_8)
    seg8_off = np.concatenate([[0], np.cumsum(seg8_rows)]).astype(int)
    C8 = int(seg8_off[-1])
    NBF = T_BF * 128

    bf16 = ml_dtypes.bfloat16
    f8 = ml_dtypes.float8_e4m3
    xb = x.astype(bf16)  # one RTN cast, reused for all bf16 gathers
    x8 = (x * np.float32(SX)).astype(f8)
    wb = [np.ascontiguousarray(expert_W[e].T.astype(bf16)) for e in range(E)]
    w8 = [
        np.ascontiguousarray((expert_W[e].T * np.float32(SW)).astype(f8))
        for e in range(E)
    ]

    # Dispatch: fill each core's xT8/xTb columns and pick per-segment
    # weights; record each pair's (core, y-row) for the combine.
    xT8s = [np.zeros((D, C8), dtype=f8) for _ in range(NCORES)]
    xTbs = [np.empty((D, NBF), dtype=bf16) for _ in range(NCORES)]
    seg8_w = [[0] * len(seg8_rows) for _ in range(NCORES)]
    core_of = np.zeros((E, B), dtype=np.int64)
    pos_of = np.zeros((E, B), dtype=np.int64)
    is8_of = np.zeros((E, B), dtype=bool)
    for e in range(E):
        tb, t8 = groups[e]
        xTbs[e][:, :] = xb[tb].T  # bf16: expert e lives on core e, full slot
        core_of[e, tb] = e
        pos_of[e, tb] = C8 + np.arange(NBF)
        cum = 0
        for c, j, rows in pieces8[e]:
            tkn = t8[cum : cum + rows]
            lo = int(seg8_off[j])
            xT8s[c][:, lo : lo + len(tkn)] = x8[tkn].T
            seg8_w[c][j] = e
            core_of[e, tkn] = c
            pos_of[e, tkn] = lo + np.arange(len(tkn))
            is8_of[e, tkn] = True
            cum += rows

    in_maps = []
    for c in range(NCORES):
        m = {"xT8": xT8s[c], "xTb": xTbs[c], "wb": wb[c]}
        for j in range(len(seg8_rows)):
            m[f"w8_{j}"] = w8[seg8_w[c][j]]
        in_maps.append(m)

    if TRACE:
        _install_profshim()
    nc = _build_bass(seg8_rows, NBF)
    res = run_bass_kernel_spmd(nc, in_maps, list(range(NCORES)), trace=TRACE)
    last_exec_time_ns = res.exec_time_ns
    if res.instructions_and_trace:
        last_trace_path = res.instructions_and_trace[1]

    Ys = np.stack([res.results[c]["y"] for c in range(NCORES)]).astype(np.float32)

    # Combine: out[b] = sum_k w_k * (Y at (core,row of (e_k, b))/scale + b_e)
    barange = np.arange(B)
    descale = np.float32(1.0 / (SX * SW))
    out = np.zeros((B, H), dtype=np.float32)
    for k in range(TOPK):
        ek = topk_idx[:, k]
        wk = topk_w[:, k]
        yv = Ys[core_of[ek, barange], pos_of[ek, barange], :]
        sc = np.where(is8_of[ek, barange], wk * descale, wk).astype(np.float32)
        out += sc[:, None] * yv + wk[:, None] * expert_b[ek]
    return out
